# revision 6
# baseline (speedup 1.0000x reference)
"""GAT (2-layer, PPI config) on 8 trn2 NeuronCores — sorted-cutoff design.

Math (per layer, head): with x = f_src[d] + f_dst[s], alpha = 0.2:
    exp(lrelu(x)) = exp(f_src[d]) exp(f_dst[s]) max(1, R[d] r[s]),
    R = exp(-0.8 f_src), r = exp(-0.8 f_dst).
Multiplying num/den by invR[d] = 1/R[d] (cancels in the softmax ratio):
    att[s,d] = adj * max(invR[d], r[s]);  raw[d] = sum_s att * whp[s],
    whp = [exp(f_dst) Wh, exp(f_dst)];  out = num/den (+elu).

Sorted-cutoff: sources sorted desc by r, destinations desc by invR.  For a
128-source tile t, columns split at host-computed cutoffs into
  [0, A[t])   "pure-invR":  att = adj * invR[d]    -> plain adj matmul into
                            acc1 (host scales by invR[d] after),
  [B[t], D)   "pure-r":     att = adj * r[s]       -> plain adj matmul vs
                            whp2 = r*whp into acc2,
  [A[t], B[t]) strip:       full gate via one fused custom-DVE op + matmul.
Bulk matmuls run fp8 DoubleRow (256-source pairs).  Cutoffs are shared
across the 8 cores (one SPMD program) by assigning each unit a strided
subset of the sorted columns, which equalizes rank profiles; A/B are the
conservative envelope over the phase's units.

Sharding: L1: 2 phases x (2 heads x 4 column-strides); L2: 1 phase,
(2 interleaved source-halves x 4 column-strides).  Host does O(N*d) prep,
sorting, normalization, elu, and un-permutation.
"""

import os
import sys

sys.path.insert(0, "/opt/trn_rl_repo")

import numpy as np
import ml_dtypes

import concourse.bass as bass
import concourse.tile as tile
from concourse import bacc, mybir
from concourse.bass_utils import run_bass_kernel_spmd

BF16 = mybir.dt.bfloat16
F32 = mybir.dt.float32
FP8 = mybir.dt.float8e4
NPBF16 = ml_dtypes.bfloat16
NPFP8 = np.dtype(mybir.dt.np(FP8))

N = 8192
NFEAT = 256
NHID = 64
NHEADS = 4
NCLASS = 121
ALPHA = 0.2
N_CORES = 8
P = 128
D = 2048            # columns per unit
WSCALE = 8.0        # fp8 stationary scale (cancels in num/den ratio)

_NC_CACHE = {}
_LAST_EXEC_NS = []
_GM = None


def _gatemask():
    """Custom DVE op: out = in1 * max(in0, s0) — fused strip gate+mask."""
    global _GM
    if _GM is not None:
        return _GM
    from concourse.dve_spec import Spec, Src0, Src1, C0, maxx, lower, _has_src1
    from concourse.dve_uop import DveOpSpec
    from concourse import dve_ops

    spec = Spec(
        body=Src1 * maxx(Src0, C0),
        reference=lambda in0, in1, s0, s1, imm2: (
            in1 * np.maximum(in0, s0)).astype(np.float32),
    )
    shas = {}
    for ver in ("v3", "v4"):
        s = DveOpSpec(name="GATEMASK_ANT", opcode=1,
                      uops=lower(spec, ver=ver), rd1_en=_has_src1(spec))
        shas[ver] = s.sha(ver)
    op = dve_ops.DveOp("GATEMASK_ANT", spec, subdim=False, uops_sha=shas)
    dve_ops.OPS.append(op)
    dve_ops.CUSTOM_DVE_SPECS[op.name] = op.spec
    dve_ops._SUB_OPCODE_FOR_NAME[op.name] = (
        dve_ops._CUSTOM_DVE_ROW_BASE + len(dve_ops.OPS) - 1)
    _GM = op
    return op


def _chunks(c0, c1, grid=512):
    """Split [c0, c1) at multiples of `grid` (PSUM bank boundaries)."""
    while c0 < c1:
        nxt = min(c1, (c0 // grid + 1) * grid)
        yield c0, nxt
        c0 = nxt


def build_sorted_kernel(phases, warmup=20):
    """phases: list of dicts with keys n_pairs, dh, A, B (tuples len 2*n_pairs).
    Inputs per phase p (per core):
      adj{p}  [n_pairs*256, D] fp8   adjacency, rows = sorted sources,
                                     cols = unit's strided sorted dests
      whp{p}  [128, n_pairs*2*128] fp8   stationary pairs (feature-major pad)
      whp2{p} [128, n_pairs*2*128] fp8   r-scaled stationary pairs
      ivb{p}  [128, D] bf16          invR broadcast (strip gate operand)
      rsc{p}  [128, 2*n_pairs] f32   r per tile (per-partition scalars)
    Output out{p} [2*(dh+1), D] f32: rows [0,dh+1) = acc1, rest = acc2."""
    gm = _gatemask()
    nc = bacc.Bacc("TRN2", target_bir_lowering=False, debug=False,
                   num_devices=N_CORES)
    ins = []
    outs = []
    for p, ph in enumerate(phases):
        npair, dh = ph["n_pairs"], ph["dh"]
        pw = ph["pw"]
        ins.append(dict(
            adj=nc.dram_tensor(f"adj{p}", [npair * 256, D], FP8,
                               kind="ExternalInput"),
            whp=nc.dram_tensor(f"whp{p}", [P, npair, 2, pw], FP8,
                               kind="ExternalInput"),
            whp2=nc.dram_tensor(f"whp2{p}", [P, npair, 2, pw], FP8,
                                kind="ExternalInput"),
            ivb=nc.dram_tensor(f"ivb{p}", [P, D], BF16,
                               kind="ExternalInput"),
            rsc=nc.dram_tensor(f"rsc{p}", [P, 2 * npair], F32,
                               kind="ExternalInput"),
        ))
        outs.append(nc.dram_tensor(f"out{p}", [dh + 1, D], F32,
                                   kind="ExternalOutput"))

    DR = mybir.MatmulPerfMode.DoubleRow
    with tile.TileContext(nc) as tc:
        with (
            tc.tile_pool(name="const", bufs=1) as cpool,
            tc.tile_pool(name="adj", bufs=8) as apool,
            tc.tile_pool(name="strip", bufs=6) as spool,
            tc.tile_pool(name="stg", bufs=4) as gpool,
            tc.tile_pool(name="acc", bufs=1,
                         space=bass.MemorySpace.PSUM) as pspool,
        ):
            acc1 = pspool.tile([P, D], F32, name="acc1")
            acc2 = pspool.tile([P, D], F32, name="acc2")

            if warmup:
                dmy = cpool.tile([P, 512], BF16)
                nc.vector.memset(dmy[:], 0.0)
                for w in range(warmup):
                    nc.tensor.matmul(acc1[:, 0:512], dmy[:, 0:P], dmy[:],
                                     start=True, stop=True,
                                     skip_group_check=True)

            for p, ph in enumerate(phases):
                npair, dh = ph["n_pairs"], ph["dh"]
                pw = ph["pw"]
                A, B = ph["A"], ph["B"]
                adj_d = ins[p]["adj"]
                whp = cpool.tile([P, npair, 2, pw], FP8, name=f"whp{p}")
                nc.sync.dma_start(whp[:], ins[p]["whp"][:])
                whp2 = cpool.tile([P, npair, 2, pw], FP8, name=f"whp2{p}")
                nc.sync.dma_start(whp2[:], ins[p]["whp2"][:])
                ivb = cpool.tile([P, D], BF16, name=f"ivb{p}")
                nc.sync.dma_start(ivb[:], ins[p]["ivb"][:])
                rsc = cpool.tile([P, 2 * npair], F32, name=f"rsc{p}")
                nc.sync.dma_start(rsc[:], ins[p]["rsc"][:])

                # zero both accumulators (ScalarE; matmuls accumulate in place)
                nc.scalar.memzero(acc1[:])
                nc.scalar.memzero(acc2[:])

                def pairw(i, kslice=None):
                    # stationary AP for pair i: [128, 2, 128] (or one k-tile)
                    if kslice is None:
                        return whp[:, i, :, :]
                    return whp[:, i, kslice, :]

                def pairw2(i, kslice=None):
                    if kslice is None:
                        return whp2[:, i, :, :]
                    return whp2[:, i, kslice, :]

                for i in range(npair):
                    t0, t1 = 2 * i, 2 * i + 1
                    A0, A1, B0, B1 = A[t0], A[t1], B[t0], B[t1]
                    adjp = apool.tile([P, 2, D], FP8, tag="adj")
                    nc.sync.dma_start(adjp[:, 0, :],
                                      adj_d[i * 256:i * 256 + P, :])
                    nc.sync.dma_start(adjp[:, 1, :],
                                      adj_d[i * 256 + P:(i + 1) * 256, :])
                    # pure-invR bulk (both tiles) -> acc1
                    for c0, c1 in _chunks(0, A0):
                        nc.tensor.matmul(acc1[0:pw, c0:c1], pairw(i),
                                         adjp[:, :, c0:c1], start=False,
                                         stop=False, perf_mode=DR,
                                         skip_group_check=True)
                    # tile t1 extra pure-invR
                    for c0, c1 in _chunks(A0, A1):
                        nc.tensor.matmul(acc1[0:pw, c0:c1], pairw(i, 1),
                                         adjp[:, 1, c0:c1], start=False,
                                         stop=False, skip_group_check=True)
                    # strips
                    for (tt, kk, sa, sb) in ((t0, 0, A0, B0), (t1, 1, A1, B1)):
                        for c0, c1 in _chunks(sa, sb):
                            g = spool.tile([P, 512], BF16, tag="strip")
                            w = c1 - c0
                            nc.vector._custom_dve(
                                gm, out=g[:, 0:w], in0=ivb[:, c0:c1],
                                in1=adjp[:, kk, c0:c1],
                                s0=rsc[:, tt:tt + 1])
                            nc.tensor.matmul(acc2[0:pw, c0:c1], pairw(i, kk),
                                             g[:, 0:w], start=False,
                                             stop=False,
                                             skip_group_check=True)
                    # tile t0 extra pure-r
                    for c0, c1 in _chunks(B0, B1):
                        nc.tensor.matmul(acc2[0:pw, c0:c1], pairw2(i, 0),
                                         adjp[:, 0, c0:c1], start=False,
                                         stop=False, skip_group_check=True)
                    # pure-r bulk -> acc2
                    for c0, c1 in _chunks(B1, D):
                        nc.tensor.matmul(acc2[0:pw, c0:c1], pairw2(i),
                                         adjp[:, :, c0:c1], start=False,
                                         stop=False, perf_mode=DR,
                                         skip_group_check=True)

                # combine raw = invR*acc1 + acc2 on-device, chunked so the
                # output DMA overlaps the DVE combine of later chunks
                for c0, c1 in _chunks(0, D):
                    w = c1 - c0
                    stg1 = gpool.tile([dh + 1, 512], F32, tag="stg")
                    nc.vector.tensor_tensor(stg1[:, 0:w],
                                            acc1[0:dh + 1, c0:c1],
                                            ivb[0:dh + 1, c0:c1],
                                            mybir.AluOpType.mult)
                    stg = gpool.tile([dh + 1, 512], F32, tag="stg")
                    nc.vector.tensor_tensor(stg[:, 0:w], stg1[:, 0:w],
                                            acc2[0:dh + 1, c0:c1],
                                            mybir.AluOpType.add)
                    for r0 in range(0, dh + 1, 32):
                        r1 = min(r0 + 32, dh + 1)
                        nc.sync.dma_start(outs[p][r0:r1, c0:c1],
                                          stg[r0:r1, 0:w])

    nc.compile()
    return nc


def _get_kernel(key, phases):
    if key not in _NC_CACHE:
        _NC_CACHE[key] = build_sorted_kernel(phases)
    return _NC_CACHE[key]


def _cutoffs(r_sorted, iv_sorted, n_tiles):
    """c1/c2 per 128-source tile vs a desc-sorted invR column vector."""
    c1 = np.empty(n_tiles, dtype=np.int64)
    c2 = np.empty(n_tiles, dtype=np.int64)
    neg = -iv_sorted
    for t in range(n_tiles):
        c1[t] = np.searchsorted(neg, -r_sorted[P * t], side="right")
        c2[t] = np.searchsorted(neg, -r_sorted[P * t + P - 1], side="left")
    return c1, c2


def _envelope(units, n_tiles):
    """units: list of (r_sorted, iv_sorted). Returns 16-snapped A, B."""
    A = np.full(n_tiles, D, dtype=np.int64)
    B = np.zeros(n_tiles, dtype=np.int64)
    for r_s, iv_s in units:
        c1, c2 = _cutoffs(r_s, iv_s, n_tiles)
        A = np.minimum(A, c1)
        B = np.maximum(B, c2)
    A = (A // 16) * 16
    B = np.minimum(-(-B // 16) * 16, D)
    B = np.maximum(B, A)
    return tuple(int(x) for x in A), tuple(int(x) for x in B)


def _prep_unit(adjT8v, sigma, cols, whp_aug, r, invR, n_pairs, pw):
    """Per-unit device inputs (adj fp8, whp/whp2 fp8 pairs, ivb, rsc)."""
    n_src = n_pairs * 256
    adj8 = adjT8v[np.ix_(sigma, cols)].view(NPFP8)      # [n_src, D] fp8
    wa = np.zeros((n_src, pw), dtype=np.float32)
    wa[:, 0:whp_aug.shape[1]] = whp_aug[sigma] / WSCALE
    w8 = np.ascontiguousarray(
        wa.reshape(n_pairs, 2, P, pw).transpose(2, 0, 1, 3))
    w28 = np.ascontiguousarray((wa * r[sigma][:, None]).reshape(
        n_pairs, 2, P, pw).transpose(2, 0, 1, 3))
    ivb = np.broadcast_to(invR[cols].astype(NPBF16)[None, :], (P, D))
    rsc = r[sigma].reshape(2 * n_pairs, P).T.astype(np.float32)
    return dict(adj=adj8, whp=w8.astype(NPFP8), whp2=w28.astype(NPFP8),
                ivb=np.ascontiguousarray(ivb), rsc=np.ascontiguousarray(rsc))


def _launch(nc, in_maps):
    trace = bool(os.environ.get("GAT_TRACE"))
    res = run_bass_kernel_spmd(nc, in_maps, list(range(N_CORES)), trace=trace)
    if trace:
        _LAST_EXEC_NS.append(res.exec_time_ns)
    return res.results


def _recombine(raw, dh):
    """Device-combined raw rows -> normalized [D, dh] block."""
    return (raw[0:dh, :] / raw[dh, :][None, :]).T   # [D, dh]


def kernel(x, adj, Ws, a_heads, W_out, a_out):
    _LAST_EXEC_NS.clear()
    x = np.asarray(x, dtype=np.float32)
    adj = np.asarray(adj, dtype=np.float32)
    Ws = np.asarray(Ws, dtype=np.float32)
    a_heads = np.asarray(a_heads, dtype=np.float32)
    W_out = np.asarray(W_out, dtype=np.float32)
    a_out = np.asarray(a_out, dtype=np.float32)

    adjT8v = np.ascontiguousarray(adj.T.astype(NPFP8)).view(np.uint8)

    # ---- Layer 1: 4 heads; phases = 2 head-pairs x 4 column strides ----
    Wh = [x @ Ws[h] for h in range(NHEADS)]
    f_src = [Wh[h] @ a_heads[h][:NHID] for h in range(NHEADS)]
    f_dst = [Wh[h] @ a_heads[h][NHID:] for h in range(NHEADS)]
    r_h = [np.exp(-(1 - ALPHA) * f_dst[h]).astype(np.float32)
           for h in range(NHEADS)]
    iv_h = [np.exp((1 - ALPHA) * f_src[h]).astype(np.float32)
            for h in range(NHEADS)]
    whp_h = [np.concatenate([np.exp(f_dst[h])[:, None] * Wh[h],
                             np.exp(f_dst[h])[:, None]], axis=1)
             for h in range(NHEADS)]
    sig_h = [np.argsort(-r_h[h], kind="stable") for h in range(NHEADS)]
    pi_h = [np.argsort(-iv_h[h], kind="stable") for h in range(NHEADS)]

    n_pairs1 = N // 256
    n_tiles1 = N // P

    # choose head pairing minimizing total envelope strip width
    def pairing_cost(ha, hb):
        units = []
        for h in (ha, hb):
            for q in range(4):
                cols = pi_h[h][q::4]
                units.append((r_h[h][sig_h[h]], iv_h[h][cols]))
        A, B = _envelope(units, n_tiles1)
        return sum(b - a for a, b in zip(A, B)), (A, B)

    best = None
    for split in (((0, 1), (2, 3)), ((0, 2), (1, 3)), ((0, 3), (1, 2))):
        c0, e0 = pairing_cost(*split[0])
        c1_, e1 = pairing_cost(*split[1])
        if best is None or c0 + c1_ < best[0]:
            best = (c0 + c1_, split, (e0, e1))
    _, split, envs = best

    phases = [dict(n_pairs=n_pairs1, dh=NHID, pw=80, A=envs[p][0],
                   B=envs[p][1]) for p in range(2)]
    key1 = ("L1", phases[0]["A"], phases[0]["B"],
            phases[1]["A"], phases[1]["B"])
    nc1 = _get_kernel(key1, phases)

    in_maps = []
    for c in range(N_CORES):
        m = {}
        for p in range(2):
            h = split[p][c // 4]
            q = c % 4
            cols = pi_h[h][q::4]
            u = _prep_unit(adjT8v, sig_h[h], cols, whp_h[h], r_h[h],
                           iv_h[h], n_pairs1, 80)
            for k, v in u.items():
                m[f"{k}{p}"] = v
        in_maps.append(m)
    results = _launch(nc1, in_maps)

    h_cat = np.empty((N, NHEADS * NHID), dtype=np.float32)
    for c in range(N_CORES):
        for p in range(2):
            h = split[p][c // 4]
            q = c % 4
            cols = pi_h[h][q::4]
            blk = _recombine(results[c][f"out{p}"], NHID)
            hb = np.where(blk > 0, blk, np.expm1(np.minimum(blk, 0)))
            h_cat[cols, h * NHID:(h + 1) * NHID] = hb

    # ---- Layer 2: 1 head; 2 interleaved source halves x 4 strides ----
    Wh2 = h_cat @ W_out
    f_src2 = Wh2 @ a_out[:NCLASS]
    f_dst2 = Wh2 @ a_out[NCLASS:]
    r2 = np.exp(-(1 - ALPHA) * f_dst2).astype(np.float32)
    iv2 = np.exp((1 - ALPHA) * f_src2).astype(np.float32)
    whp2_aug = np.concatenate([np.exp(f_dst2)[:, None] * Wh2,
                               np.exp(f_dst2)[:, None]], axis=1)
    sig2 = np.argsort(-r2, kind="stable")
    pi2 = np.argsort(-iv2, kind="stable")
    halves = [sig2[0::2], sig2[1::2]]       # interleaved: matched profiles
    n_pairs2 = (N // 2) // 256
    n_tiles2 = (N // 2) // P

    units2 = []
    for H in range(2):
        for q in range(4):
            cols = pi2[q::4]
            units2.append((r2[halves[H]], iv2[cols]))
    A2, B2 = _envelope(units2, n_tiles2)
    phases2 = [dict(n_pairs=n_pairs2, dh=NCLASS, pw=128, A=A2, B=B2)]
    key2 = ("L2", A2, B2)
    nc2 = _get_kernel(key2, phases2)

    in_maps2 = []
    for c in range(N_CORES):
        H, q = c // 4, c % 4
        cols = pi2[q::4]
        u = _prep_unit(adjT8v, halves[H], cols, whp2_aug, r2, iv2,
                       n_pairs2, 128)
        in_maps2.append({f"{k}0": v for k, v in u.items()})
    results2 = _launch(nc2, in_maps2)

    out = np.empty((N, NCLASS), dtype=np.float32)
    for q in range(4):
        cols = pi2[q::4]
        o = results2[q]["out0"] + results2[q + 4]["out0"]
        out[cols, :] = _recombine(o, NCLASS)
    return out


# revision 7
# speedup vs baseline: 1.0041x; 1.0041x over previous
"""GAT (2-layer, PPI config) on 8 trn2 NeuronCores — sorted-cutoff design.

Math (per layer, head): with x = f_src[d] + f_dst[s], alpha = 0.2:
    exp(lrelu(x)) = exp(f_src[d]) exp(f_dst[s]) max(1, R[d] r[s]),
    R = exp(-0.8 f_src), r = exp(-0.8 f_dst).
Multiplying num/den by invR[d] = 1/R[d] (cancels in the softmax ratio):
    att[s,d] = adj * max(invR[d], r[s]);  raw[d] = sum_s att * whp[s],
    whp = [exp(f_dst) Wh, exp(f_dst)];  out = num/den (+elu).

Sorted-cutoff: sources sorted desc by r, destinations desc by invR.  For a
128-source tile t, columns split at host-computed cutoffs into
  [0, A[t])   "pure-invR":  att = adj * invR[d]    -> plain adj matmul into
                            acc1 (host scales by invR[d] after),
  [B[t], D)   "pure-r":     att = adj * r[s]       -> plain adj matmul vs
                            whp2 = r*whp into acc2,
  [A[t], B[t]) strip:       full gate via one fused custom-DVE op + matmul.
Bulk matmuls run fp8 DoubleRow (256-source pairs).  Cutoffs are shared
across the 8 cores (one SPMD program) by assigning each unit a strided
subset of the sorted columns, which equalizes rank profiles; A/B are the
conservative envelope over the phase's units.

Sharding: L1: 2 phases x (2 heads x 4 column-strides); L2: 1 phase,
(2 interleaved source-halves x 4 column-strides).  Host does O(N*d) prep,
sorting, normalization, elu, and un-permutation.
"""

import os
import sys

sys.path.insert(0, "/opt/trn_rl_repo")

import numpy as np
import ml_dtypes

import concourse.bass as bass
import concourse.tile as tile
from concourse import bacc, mybir
from concourse.bass_utils import run_bass_kernel_spmd

BF16 = mybir.dt.bfloat16
F32 = mybir.dt.float32
FP8 = mybir.dt.float8e4
NPBF16 = ml_dtypes.bfloat16
NPFP8 = np.dtype(mybir.dt.np(FP8))

N = 8192
NFEAT = 256
NHID = 64
NHEADS = 4
NCLASS = 121
ALPHA = 0.2
N_CORES = 8
P = 128
D = 2048            # columns per unit
WSCALE = 8.0        # fp8 stationary scale (cancels in num/den ratio)

_NC_CACHE = {}
_LAST_EXEC_NS = []
_GM = None


def _gatemask():
    """Custom DVE op: out = in1 * max(in0, s0) — fused strip gate+mask."""
    global _GM
    if _GM is not None:
        return _GM
    from concourse.dve_spec import Spec, Src0, Src1, C0, maxx, lower, _has_src1
    from concourse.dve_uop import DveOpSpec
    from concourse import dve_ops

    spec = Spec(
        body=Src1 * maxx(Src0, C0),
        reference=lambda in0, in1, s0, s1, imm2: (
            in1 * np.maximum(in0, s0)).astype(np.float32),
    )
    shas = {}
    for ver in ("v3", "v4"):
        s = DveOpSpec(name="GATEMASK_ANT", opcode=1,
                      uops=lower(spec, ver=ver), rd1_en=_has_src1(spec))
        shas[ver] = s.sha(ver)
    op = dve_ops.DveOp("GATEMASK_ANT", spec, subdim=False, uops_sha=shas)
    dve_ops.OPS.append(op)
    dve_ops.CUSTOM_DVE_SPECS[op.name] = op.spec
    dve_ops._SUB_OPCODE_FOR_NAME[op.name] = (
        dve_ops._CUSTOM_DVE_ROW_BASE + len(dve_ops.OPS) - 1)
    _GM = op
    return op


def _chunks(c0, c1, grid=512):
    """Split [c0, c1) at multiples of `grid` (PSUM bank boundaries)."""
    while c0 < c1:
        nxt = min(c1, (c0 // grid + 1) * grid)
        yield c0, nxt
        c0 = nxt


def build_sorted_kernel(phases, warmup=20):
    """phases: list of dicts with keys n_pairs, dh, A, B (tuples len 2*n_pairs).
    Inputs per phase p (per core):
      adj{p}  [n_pairs*256, D] fp8   adjacency, rows = sorted sources,
                                     cols = unit's strided sorted dests
      whp{p}  [128, n_pairs*2*128] fp8   stationary pairs (feature-major pad)
      whp2{p} [128, n_pairs*2*128] fp8   r-scaled stationary pairs
      ivb{p}  [128, D] bf16          invR broadcast (strip gate operand)
      rsc{p}  [128, 2*n_pairs] f32   r per tile (per-partition scalars)
    Output out{p} [2*(dh+1), D] f32: rows [0,dh+1) = acc1, rest = acc2."""
    gm = _gatemask()
    nc = bacc.Bacc("TRN2", target_bir_lowering=False, debug=False,
                   num_devices=N_CORES)
    ins = []
    outs = []
    for p, ph in enumerate(phases):
        npair, dh = ph["n_pairs"], ph["dh"]
        pw = ph["pw"]
        ins.append(dict(
            adj=nc.dram_tensor(f"adj{p}", [npair * 256, D], FP8,
                               kind="ExternalInput"),
            whp=nc.dram_tensor(f"whp{p}", [P, npair, 2, pw], FP8,
                               kind="ExternalInput"),
            whp2=nc.dram_tensor(f"whp2{p}", [P, npair, 2, pw], FP8,
                                kind="ExternalInput"),
            ivb=nc.dram_tensor(f"ivb{p}", [P, D], BF16,
                               kind="ExternalInput"),
            rsc=nc.dram_tensor(f"rsc{p}", [P, 2 * npair], F32,
                               kind="ExternalInput"),
        ))
        outs.append(nc.dram_tensor(f"out{p}", [dh + 1, D], F32,
                                   kind="ExternalOutput"))

    DR = mybir.MatmulPerfMode.DoubleRow
    with tile.TileContext(nc) as tc:
        with (
            tc.tile_pool(name="const", bufs=1) as cpool,
            tc.tile_pool(name="adj", bufs=8) as apool,
            tc.tile_pool(name="strip", bufs=6) as spool,
            tc.tile_pool(name="stg", bufs=4) as gpool,
            tc.tile_pool(name="acc", bufs=1,
                         space=bass.MemorySpace.PSUM) as pspool,
        ):
            acc1 = pspool.tile([P, D], F32, name="acc1")
            acc2 = pspool.tile([P, D], F32, name="acc2")

            if warmup:
                dmy = cpool.tile([P, 512], BF16)
                nc.vector.memset(dmy[:], 0.0)
                for w in range(warmup):
                    nc.tensor.matmul(acc1[:, 0:512], dmy[:, 0:P], dmy[:],
                                     start=True, stop=True,
                                     skip_group_check=True)

            for p, ph in enumerate(phases):
                npair, dh = ph["n_pairs"], ph["dh"]
                pw = ph["pw"]
                A, B = ph["A"], ph["B"]
                adj_d = ins[p]["adj"]
                whp = cpool.tile([P, npair, 2, pw], FP8, name=f"whp{p}")
                nc.sync.dma_start(whp[:], ins[p]["whp"][:])
                whp2 = cpool.tile([P, npair, 2, pw], FP8, name=f"whp2{p}")
                nc.sync.dma_start(whp2[:], ins[p]["whp2"][:])
                ivb = cpool.tile([P, D], BF16, name=f"ivb{p}")
                nc.sync.dma_start(ivb[:], ins[p]["ivb"][:])
                rsc = cpool.tile([P, 2 * npair], F32, name=f"rsc{p}")
                nc.sync.dma_start(rsc[:], ins[p]["rsc"][:])

                # zero both accumulators (ScalarE; matmuls accumulate in place)
                nc.scalar.memzero(acc1[:])
                nc.scalar.memzero(acc2[:])

                def pairw(i, kslice=None):
                    # stationary AP for pair i: [128, 2, 128] (or one k-tile)
                    if kslice is None:
                        return whp[:, i, :, :]
                    return whp[:, i, kslice, :]

                def pairw2(i, kslice=None):
                    if kslice is None:
                        return whp2[:, i, :, :]
                    return whp2[:, i, kslice, :]

                for i in range(npair):
                    t0, t1 = 2 * i, 2 * i + 1
                    A0, A1, B0, B1 = A[t0], A[t1], B[t0], B[t1]
                    adjp = apool.tile([P, 2, D], FP8, tag="adj")
                    nc.sync.dma_start(adjp[:, 0, :],
                                      adj_d[i * 256:i * 256 + P, :])
                    nc.sync.dma_start(adjp[:, 1, :],
                                      adj_d[i * 256 + P:(i + 1) * 256, :])
                    # pure-invR bulk (both tiles) -> acc1
                    for c0, c1 in _chunks(0, A0):
                        nc.tensor.matmul(acc1[0:pw, c0:c1], pairw(i),
                                         adjp[:, :, c0:c1], start=False,
                                         stop=False, perf_mode=DR,
                                         skip_group_check=True)
                    # tile t1 extra pure-invR
                    for c0, c1 in _chunks(A0, A1):
                        nc.tensor.matmul(acc1[0:pw, c0:c1], pairw(i, 1),
                                         adjp[:, 1, c0:c1], start=False,
                                         stop=False, skip_group_check=True)
                    # strips
                    for (tt, kk, sa, sb) in ((t0, 0, A0, B0), (t1, 1, A1, B1)):
                        for c0, c1 in _chunks(sa, sb):
                            g = spool.tile([P, 512], BF16, tag="strip")
                            w = c1 - c0
                            nc.vector._custom_dve(
                                gm, out=g[:, 0:w], in0=ivb[:, c0:c1],
                                in1=adjp[:, kk, c0:c1],
                                s0=rsc[:, tt:tt + 1])
                            nc.tensor.matmul(acc2[0:pw, c0:c1], pairw(i, kk),
                                             g[:, 0:w], start=False,
                                             stop=False,
                                             skip_group_check=True)
                    # tile t0 extra pure-r
                    for c0, c1 in _chunks(B0, B1):
                        nc.tensor.matmul(acc2[0:pw, c0:c1], pairw2(i, 0),
                                         adjp[:, 0, c0:c1], start=False,
                                         stop=False, skip_group_check=True)
                    # pure-r bulk -> acc2
                    for c0, c1 in _chunks(B1, D):
                        nc.tensor.matmul(acc2[0:pw, c0:c1], pairw2(i),
                                         adjp[:, :, c0:c1], start=False,
                                         stop=False, perf_mode=DR,
                                         skip_group_check=True)

                # combine raw = invR*acc1 + acc2 on-device, then DMA out
                stg1 = gpool.tile([dh + 1, D], F32, tag="stg")
                nc.vector.tensor_tensor(stg1[:], acc1[0:dh + 1, :],
                                        ivb[0:dh + 1, :],
                                        mybir.AluOpType.mult)
                stg = gpool.tile([dh + 1, D], F32, tag="stg")
                nc.vector.tensor_tensor(stg[:], stg1[:], acc2[0:dh + 1, :],
                                        mybir.AluOpType.add)
                for c0 in range(0, dh + 1, 16):
                    c1 = min(c0 + 16, dh + 1)
                    nc.sync.dma_start(outs[p][c0:c1, :], stg[c0:c1, :])

    nc.compile()
    return nc


def _get_kernel(key, phases):
    if key not in _NC_CACHE:
        _NC_CACHE[key] = build_sorted_kernel(phases)
    return _NC_CACHE[key]


def _cutoffs(r_sorted, iv_sorted, n_tiles):
    """c1/c2 per 128-source tile vs a desc-sorted invR column vector."""
    c1 = np.empty(n_tiles, dtype=np.int64)
    c2 = np.empty(n_tiles, dtype=np.int64)
    neg = -iv_sorted
    for t in range(n_tiles):
        c1[t] = np.searchsorted(neg, -r_sorted[P * t], side="right")
        c2[t] = np.searchsorted(neg, -r_sorted[P * t + P - 1], side="left")
    return c1, c2


def _envelope(units, n_tiles):
    """units: list of (r_sorted, iv_sorted). Returns 16-snapped A, B."""
    A = np.full(n_tiles, D, dtype=np.int64)
    B = np.zeros(n_tiles, dtype=np.int64)
    for r_s, iv_s in units:
        c1, c2 = _cutoffs(r_s, iv_s, n_tiles)
        A = np.minimum(A, c1)
        B = np.maximum(B, c2)
    A = (A // 16) * 16
    B = np.minimum(-(-B // 16) * 16, D)
    B = np.maximum(B, A)
    return tuple(int(x) for x in A), tuple(int(x) for x in B)


def _prep_unit(adjT8v, sigma, cols, whp_aug, r, invR, n_pairs, pw):
    """Per-unit device inputs (adj fp8, whp/whp2 fp8 pairs, ivb, rsc)."""
    n_src = n_pairs * 256
    adj8 = adjT8v[np.ix_(sigma, cols)].view(NPFP8)      # [n_src, D] fp8
    wa = np.zeros((n_src, pw), dtype=np.float32)
    wa[:, 0:whp_aug.shape[1]] = whp_aug[sigma] / WSCALE
    w8 = np.ascontiguousarray(
        wa.reshape(n_pairs, 2, P, pw).transpose(2, 0, 1, 3))
    w28 = np.ascontiguousarray((wa * r[sigma][:, None]).reshape(
        n_pairs, 2, P, pw).transpose(2, 0, 1, 3))
    ivb = np.broadcast_to(invR[cols].astype(NPBF16)[None, :], (P, D))
    rsc = r[sigma].reshape(2 * n_pairs, P).T.astype(np.float32)
    return dict(adj=adj8, whp=w8.astype(NPFP8), whp2=w28.astype(NPFP8),
                ivb=np.ascontiguousarray(ivb), rsc=np.ascontiguousarray(rsc))


def _launch(nc, in_maps):
    trace = bool(os.environ.get("GAT_TRACE"))
    res = run_bass_kernel_spmd(nc, in_maps, list(range(N_CORES)), trace=trace)
    if trace:
        _LAST_EXEC_NS.append(res.exec_time_ns)
    return res.results


def _recombine(raw, dh):
    """Device-combined raw rows -> normalized [D, dh] block."""
    return (raw[0:dh, :] / raw[dh, :][None, :]).T   # [D, dh]


def kernel(x, adj, Ws, a_heads, W_out, a_out):
    _LAST_EXEC_NS.clear()
    x = np.asarray(x, dtype=np.float32)
    adj = np.asarray(adj, dtype=np.float32)
    Ws = np.asarray(Ws, dtype=np.float32)
    a_heads = np.asarray(a_heads, dtype=np.float32)
    W_out = np.asarray(W_out, dtype=np.float32)
    a_out = np.asarray(a_out, dtype=np.float32)

    adjT8v = np.ascontiguousarray(adj.T.astype(NPFP8)).view(np.uint8)

    # ---- Layer 1: 4 heads; phases = 2 head-pairs x 4 column strides ----
    Wh = [x @ Ws[h] for h in range(NHEADS)]
    f_src = [Wh[h] @ a_heads[h][:NHID] for h in range(NHEADS)]
    f_dst = [Wh[h] @ a_heads[h][NHID:] for h in range(NHEADS)]
    r_h = [np.exp(-(1 - ALPHA) * f_dst[h]).astype(np.float32)
           for h in range(NHEADS)]
    iv_h = [np.exp((1 - ALPHA) * f_src[h]).astype(np.float32)
            for h in range(NHEADS)]
    whp_h = [np.concatenate([np.exp(f_dst[h])[:, None] * Wh[h],
                             np.exp(f_dst[h])[:, None]], axis=1)
             for h in range(NHEADS)]
    sig_h = [np.argsort(-r_h[h], kind="stable") for h in range(NHEADS)]
    pi_h = [np.argsort(-iv_h[h], kind="stable") for h in range(NHEADS)]

    n_pairs1 = N // 256
    n_tiles1 = N // P

    # choose head pairing minimizing total envelope strip width
    def pairing_cost(ha, hb):
        units = []
        for h in (ha, hb):
            for q in range(4):
                cols = pi_h[h][q::4]
                units.append((r_h[h][sig_h[h]], iv_h[h][cols]))
        A, B = _envelope(units, n_tiles1)
        return sum(b - a for a, b in zip(A, B)), (A, B)

    best = None
    for split in (((0, 1), (2, 3)), ((0, 2), (1, 3)), ((0, 3), (1, 2))):
        c0, e0 = pairing_cost(*split[0])
        c1_, e1 = pairing_cost(*split[1])
        if best is None or c0 + c1_ < best[0]:
            best = (c0 + c1_, split, (e0, e1))
    _, split, envs = best

    phases = [dict(n_pairs=n_pairs1, dh=NHID, pw=80, A=envs[p][0],
                   B=envs[p][1]) for p in range(2)]
    key1 = ("L1", phases[0]["A"], phases[0]["B"],
            phases[1]["A"], phases[1]["B"])
    nc1 = _get_kernel(key1, phases)

    in_maps = []
    for c in range(N_CORES):
        m = {}
        for p in range(2):
            h = split[p][c // 4]
            q = c % 4
            cols = pi_h[h][q::4]
            u = _prep_unit(adjT8v, sig_h[h], cols, whp_h[h], r_h[h],
                           iv_h[h], n_pairs1, 80)
            for k, v in u.items():
                m[f"{k}{p}"] = v
        in_maps.append(m)
    results = _launch(nc1, in_maps)

    h_cat = np.empty((N, NHEADS * NHID), dtype=np.float32)
    for c in range(N_CORES):
        for p in range(2):
            h = split[p][c // 4]
            q = c % 4
            cols = pi_h[h][q::4]
            blk = _recombine(results[c][f"out{p}"], NHID)
            hb = np.where(blk > 0, blk, np.expm1(np.minimum(blk, 0)))
            h_cat[cols, h * NHID:(h + 1) * NHID] = hb

    # ---- Layer 2: 1 head; 2 interleaved source halves x 4 strides ----
    Wh2 = h_cat @ W_out
    f_src2 = Wh2 @ a_out[:NCLASS]
    f_dst2 = Wh2 @ a_out[NCLASS:]
    r2 = np.exp(-(1 - ALPHA) * f_dst2).astype(np.float32)
    iv2 = np.exp((1 - ALPHA) * f_src2).astype(np.float32)
    whp2_aug = np.concatenate([np.exp(f_dst2)[:, None] * Wh2,
                               np.exp(f_dst2)[:, None]], axis=1)
    sig2 = np.argsort(-r2, kind="stable")
    pi2 = np.argsort(-iv2, kind="stable")
    halves = [sig2[0::2], sig2[1::2]]       # interleaved: matched profiles
    n_pairs2 = (N // 2) // 256
    n_tiles2 = (N // 2) // P

    units2 = []
    for H in range(2):
        for q in range(4):
            cols = pi2[q::4]
            units2.append((r2[halves[H]], iv2[cols]))
    A2, B2 = _envelope(units2, n_tiles2)
    phases2 = [dict(n_pairs=n_pairs2, dh=NCLASS, pw=128, A=A2, B=B2)]
    key2 = ("L2", A2, B2)
    nc2 = _get_kernel(key2, phases2)

    in_maps2 = []
    for c in range(N_CORES):
        H, q = c // 4, c % 4
        cols = pi2[q::4]
        u = _prep_unit(adjT8v, halves[H], cols, whp2_aug, r2, iv2,
                       n_pairs2, 128)
        in_maps2.append({f"{k}0": v for k, v in u.items()})
    results2 = _launch(nc2, in_maps2)

    out = np.empty((N, NCLASS), dtype=np.float32)
    for q in range(4):
        cols = pi2[q::4]
        o = results2[q]["out0"] + results2[q + 4]["out0"]
        out[cols, :] = _recombine(o, NCLASS)
    return out
